# revision 8
# baseline (speedup 1.0000x reference)
import numpy as np

E, D, F, T = 8, 1024, 2048, 4096
JITTER_EPS = 0.01
TP_DEFAULT = 1088

_CACHE = {}


def _token_tiles(tp):
    # split tp into chunks <=512, each >=256 (full-rate f32r)
    tiles = []
    off = 0
    rem = tp
    while rem > 0:
        if rem > 512 and rem - 512 < 256:
            sz = rem - 256
        else:
            sz = min(512, rem)
        tiles.append((off, sz))
        off += sz
        rem -= sz
    return tiles


def _build(tp):
    from concourse import bacc, mybir, tile

    tiles = _token_tiles(tp)
    nc = bacc.Bacc("TRN2", target_bir_lowering=False, debug=False, num_devices=8)
    f32 = mybir.dt.float32
    f32r = mybir.dt.float32r
    xg = nc.dram_tensor("xg", [8, 128, tp], f32r, kind="ExternalInput").ap()
    wg = nc.dram_tensor("wg", [16, 128, 1024], f32r, kind="ExternalInput").ap()
    wu = nc.dram_tensor("wu", [16, 128, 1024], f32r, kind="ExternalInput").ap()
    wd = nc.dram_tensor("wd", [8, 128, 2048], f32r, kind="ExternalInput").ap()
    cb = nc.dram_tensor("cb", [128, tp], f32, kind="ExternalInput").ap()
    y = nc.dram_tensor("y", [1024, tp], f32, kind="ExternalOutput").ap()

    with tile.TileContext(nc) as tc:
        with tc.tile_pool(name="xp", bufs=1) as xp, \
             tc.tile_pool(name="wp", bufs=2) as wp, \
             tc.tile_pool(name="wdp", bufs=2) as wdp, \
             tc.tile_pool(name="hp", bufs=1) as hp, \
             tc.tile_pool(name="cp", bufs=1) as cp, \
             tc.tile_pool(name="sp", bufs=2) as spool, \
             tc.tile_pool(name="yp", bufs=2) as yp, \
             tc.tile_pool(name="psA", bufs=2, space="PSUM") as psA, \
             tc.tile_pool(name="psB", bufs=2, space="PSUM") as psB:
            x_sb = []
            for k in range(8):
                t_ = xp.tile([128, tp], f32r, name=f"x_{k}")
                nc.sync.dma_start(t_[:], xg[k, :, :])
                x_sb.append(t_)
            cb_sb = cp.tile([128, tp], f32, name="cb_sb")
            nc.sync.dma_start(cb_sb[:], cb[:, :])
            h_sb = [hp.tile([128, tp], f32r, name=f"h_{i}") for i in range(16)]

            for ft in range(16):
                wgt = wp.tile([128, 1024], f32r, name="wgt")
                wut = wp.tile([128, 1024], f32r, name="wut")
                nc.sync.dma_start(wgt[:], wg[ft, :, :])
                nc.sync.dma_start(wut[:], wu[ft, :, :])
                for (toff, tsz) in tiles:
                    pg = psA.tile([128, 512], f32, name="pg")
                    pu = psA.tile([128, 512], f32, name="pu")
                    for k in range(8):
                        nc.tensor.matmul(pg[:, :tsz], wgt[:, k * 128:(k + 1) * 128],
                                         x_sb[k][:, toff:toff + tsz],
                                         start=(k == 0), stop=(k == 7))
                    for k in range(8):
                        nc.tensor.matmul(pu[:, :tsz], wut[:, k * 128:(k + 1) * 128],
                                         x_sb[k][:, toff:toff + tsz],
                                         start=(k == 0), stop=(k == 7))
                    sg = spool.tile([128, 512], f32, name="sg")
                    nc.scalar.activation(sg[:, :tsz], pg[:, :tsz],
                                         mybir.ActivationFunctionType.Sigmoid)
                    nc.vector.tensor_tensor(sg[:, :tsz], sg[:, :tsz], pg[:, :tsz],
                                            mybir.AluOpType.mult)
                    nc.vector.tensor_tensor(h_sb[ft][:, toff:toff + tsz], sg[:, :tsz],
                                            pu[:, :tsz], mybir.AluOpType.mult)

            for dt_ in range(8):
                wdt = wdp.tile([128, 2048], f32r, name="wdt")
                nc.sync.dma_start(wdt[:], wd[dt_, :, :])
                for (toff, tsz) in tiles:
                    py_ = psB.tile([128, 512], f32, name="py")
                    for k2 in range(16):
                        nc.tensor.matmul(py_[:, :tsz], wdt[:, k2 * 128:(k2 + 1) * 128],
                                         h_sb[k2][:, toff:toff + tsz],
                                         start=(k2 == 0), stop=(k2 == 15))
                    ysb = yp.tile([128, 512], f32, name="ysb")
                    nc.vector.tensor_tensor(ysb[:, :tsz], py_[:, :tsz],
                                            cb_sb[:, toff:toff + tsz],
                                            mybir.AluOpType.mult)
                    nc.sync.dma_start(y[dt_ * 128:(dt_ + 1) * 128, toff:toff + tsz],
                                      ysb[:, :tsz])
    nc.compile()
    return nc


def _get_nc(tp):
    if tp not in _CACHE:
        _CACHE[tp] = _build(tp)
    return _CACHE[tp]


def _route(x, gate_w):
    logits = x @ gate_w.T  # [T, E] fp32
    with np.errstate(all="ignore"):
        max1 = logits.max(-1, keepdims=True)
        ind1 = logits.argmax(-1)
        factor1 = np.maximum(np.abs(logits), max1)
        mask1 = (max1 - logits) / factor1 > 2.0 * JITTER_EPS
        m1 = np.where(mask1, -np.inf, logits)
        e1 = np.exp(m1 - max1)
        g1 = e1 / e1.sum(-1, keepdims=True)
        mult1 = np.take_along_axis(g1, ind1[:, None], -1)[:, 0]
        oh1 = np.arange(E)[None, :] == ind1[:, None]
        ms = np.where(oh1, -np.inf, logits)
        max2 = ms.max(-1, keepdims=True)
        ind2 = ms.argmax(-1)
        factor2 = np.maximum(np.abs(logits), max2)
        mask2 = (max2 - logits) / factor2 > 2.0 * JITTER_EPS
        m2 = np.where(mask2, -np.inf, ms)
        e2 = np.exp(m2 - max2)
        g2 = e2 / e2.sum(-1, keepdims=True)
        mult2 = np.take_along_axis(g2, ind2[:, None], -1)[:, 0]
    return logits, ind1, mult1.astype(np.float32), ind2, mult2.astype(np.float32)


def _tile_weights(w_gate_e, w_up_e, w_down_e):
    # wg/wu: [F, D] -> [16(ft), 128(p), 8(k), 128(c)]; tile[p, k*128+c] = w[ft*128+c, k*128+p]
    wgt = np.ascontiguousarray(
        w_gate_e.reshape(16, 128, 8, 128).transpose(0, 3, 2, 1)).reshape(16, 128, 1024)
    wut = np.ascontiguousarray(
        w_up_e.reshape(16, 128, 8, 128).transpose(0, 3, 2, 1)).reshape(16, 128, 1024)
    # wd: [D, F] -> [8(dt), 128(p), 16(k2), 128(c)]; tile[p, k2*128+c] = w[dt*128+c, k2*128+p]
    wdt = np.ascontiguousarray(
        w_down_e.reshape(8, 128, 16, 128).transpose(0, 3, 2, 1)).reshape(8, 128, 2048)
    return wgt, wut, wdt


def kernel(hidden_states, gate_w, w_gate, w_up, w_down):
    from concourse.bass_utils import run_bass_kernel_spmd

    x = np.ascontiguousarray(hidden_states.reshape(-1, D)).astype(np.float32, copy=False)
    logits, ind1, mult1, ind2, mult2 = _route(x, gate_w.astype(np.float32, copy=False))

    per_core_idx = []
    per_core_w = []
    pos1 = np.empty(T, np.int64)
    pos2 = np.empty(T, np.int64)
    for e in range(E):
        t1 = np.nonzero(ind1 == e)[0]
        t2 = np.nonzero(ind2 == e)[0]
        pos1[t1] = np.arange(len(t1))
        pos2[t2] = len(t1) + np.arange(len(t2))
        per_core_idx.append(np.concatenate([t1, t2]))
        per_core_w.append(np.concatenate([mult1[t1], mult2[t2]]))

    max_cnt = max(len(i) for i in per_core_idx)
    tp = TP_DEFAULT
    if max_cnt > tp:
        tp = -(-max_cnt // 256) * 256
    nc = _get_nc(tp)

    in_maps = []
    for e in range(E):
        idx = per_core_idx[e]
        n = len(idx)
        xg_full = np.zeros((tp, D), np.float32)
        xg_full[:n] = x[idx]
        x_tiled = np.ascontiguousarray(xg_full.reshape(tp, 8, 128).transpose(1, 2, 0))
        cbv = np.zeros(tp, np.float32)
        cbv[:n] = per_core_w[e]
        cb_tiled = np.ascontiguousarray(np.broadcast_to(cbv, (128, tp)))
        wgt, wut, wdt = _tile_weights(
            np.asarray(w_gate[e], np.float32),
            np.asarray(w_up[e], np.float32),
            np.asarray(w_down[e], np.float32))
        in_maps.append({"xg": x_tiled, "wg": wgt, "wu": wut, "wd": wdt, "cb": cb_tiled})

    res = run_bass_kernel_spmd(nc, in_maps, list(range(8)))
    Y = np.stack([np.asarray(res.results[e]["y"]) for e in range(E)])  # [E, D, tp]
    out = Y[ind1, :, pos1] + Y[ind2, :, pos2]  # [T, D]
    return (np.ascontiguousarray(out.reshape(2, 2048, D), dtype=np.float32),
            logits.astype(np.float32, copy=False))


# revision 9
# speedup vs baseline: 1.1124x; 1.1124x over previous
import numpy as np

E, D, F, T = 8, 1024, 2048, 4096
JITTER_EPS = 0.01
TP_DEFAULT = 1024
CAP = 1024

_CACHE = {}


def _token_tiles(tp):
    tiles = []
    off = 0
    rem = tp
    while rem > 0:
        if rem > 512 and rem - 512 < 256:
            sz = rem - 256
        else:
            sz = min(512, rem)
        tiles.append((off, sz))
        off += sz
        rem -= sz
    return tiles


def _build(tp):
    from concourse import bacc, mybir, tile

    tiles = _token_tiles(tp)
    nc = bacc.Bacc("TRN2", target_bir_lowering=False, debug=False, num_devices=8)
    f32 = mybir.dt.float32
    f32r = mybir.dt.float32r
    xg = nc.dram_tensor("xg", [8, 128, tp], f32r, kind="ExternalInput").ap()
    wg = nc.dram_tensor("wg", [16, 128, 1024], f32r, kind="ExternalInput").ap()
    wu = nc.dram_tensor("wu", [16, 128, 1024], f32r, kind="ExternalInput").ap()
    wd = nc.dram_tensor("wd", [8, 128, 2048], f32r, kind="ExternalInput").ap()
    cb = nc.dram_tensor("cb", [128, tp], f32, kind="ExternalInput").ap()
    y = nc.dram_tensor("y", [1024, tp], f32, kind="ExternalOutput").ap()

    with tile.TileContext(nc) as tc:
        with tc.tile_pool(name="xp", bufs=1) as xp, \
             tc.tile_pool(name="wp", bufs=2) as wp, \
             tc.tile_pool(name="wdp", bufs=2) as wdp, \
             tc.tile_pool(name="hp", bufs=1) as hp, \
             tc.tile_pool(name="cp", bufs=1) as cp, \
             tc.tile_pool(name="sp", bufs=2) as spool, \
             tc.tile_pool(name="yp", bufs=2) as yp, \
             tc.tile_pool(name="psA", bufs=2, space="PSUM") as psA, \
             tc.tile_pool(name="psB", bufs=2, space="PSUM") as psB:
            # first gate/up weight tiles go out on the sync queue before x
            wgt_c = wp.tile([128, 1024], f32r, name="wgt")
            wut_c = wp.tile([128, 1024], f32r, name="wut")
            nc.sync.dma_start(wgt_c[:], wg[0, :, :])
            nc.sync.dma_start(wut_c[:], wu[0, :, :])

            # x and cb stream on the activation queue in parallel
            x_sb = []
            for k in range(8):
                t_ = xp.tile([128, tp], f32r, name=f"x_{k}")
                nc.scalar.dma_start(t_[:], xg[k, :, :])
                x_sb.append(t_)
            cb_sb = cp.tile([128, tp], f32, name="cb_sb")
            nc.scalar.dma_start(cb_sb[:], cb[:, :])

            h_sb = [hp.tile([128, tp], f32r, name=f"h_{i}") for i in range(16)]

            for ft in range(16):
                wgt, wut = wgt_c, wut_c
                if ft + 1 < 16:
                    wgt_c = wp.tile([128, 1024], f32r, name="wgt")
                    wut_c = wp.tile([128, 1024], f32r, name="wut")
                    nc.sync.dma_start(wgt_c[:], wg[ft + 1, :, :])
                    nc.sync.dma_start(wut_c[:], wu[ft + 1, :, :])
                for (toff, tsz) in tiles:
                    pg = psA.tile([128, 512], f32, name="pg")
                    pu = psA.tile([128, 512], f32, name="pu")
                    for k in range(8):
                        nc.tensor.matmul(pg[:, :tsz], wgt[:, k * 128:(k + 1) * 128],
                                         x_sb[k][:, toff:toff + tsz],
                                         start=(k == 0), stop=(k == 7))
                    for k in range(8):
                        nc.tensor.matmul(pu[:, :tsz], wut[:, k * 128:(k + 1) * 128],
                                         x_sb[k][:, toff:toff + tsz],
                                         start=(k == 0), stop=(k == 7))
                    sg = spool.tile([128, 512], f32, name="sg")
                    nc.scalar.activation(sg[:, :tsz], pg[:, :tsz],
                                         mybir.ActivationFunctionType.Sigmoid)
                    nc.vector.tensor_tensor(sg[:, :tsz], sg[:, :tsz], pg[:, :tsz],
                                            mybir.AluOpType.mult)
                    nc.vector.tensor_tensor(h_sb[ft][:, toff:toff + tsz], sg[:, :tsz],
                                            pu[:, :tsz], mybir.AluOpType.mult)

            wdt_c = wdp.tile([128, 2048], f32r, name="wdt")
            nc.sync.dma_start(wdt_c[:], wd[0, :, :])
            for dt_ in range(8):
                wdt = wdt_c
                if dt_ + 1 < 8:
                    wdt_c = wdp.tile([128, 2048], f32r, name="wdt")
                    nc.sync.dma_start(wdt_c[:], wd[dt_ + 1, :, :])
                for (toff, tsz) in tiles:
                    py_ = psB.tile([128, 512], f32, name="py")
                    for k2 in range(16):
                        nc.tensor.matmul(py_[:, :tsz], wdt[:, k2 * 128:(k2 + 1) * 128],
                                         h_sb[k2][:, toff:toff + tsz],
                                         start=(k2 == 0), stop=(k2 == 15))
                    ysb = yp.tile([128, 512], f32, name="ysb")
                    nc.vector.tensor_tensor(ysb[:, :tsz], py_[:, :tsz],
                                            cb_sb[:, toff:toff + tsz],
                                            mybir.AluOpType.mult)
                    nc.scalar.dma_start(y[dt_ * 128:(dt_ + 1) * 128, toff:toff + tsz],
                                        ysb[:, :tsz])
    nc.compile()
    return nc


def _get_nc(tp):
    if tp not in _CACHE:
        _CACHE[tp] = _build(tp)
    return _CACHE[tp]


def _route(x, gate_w):
    logits = x @ gate_w.T  # [T, E] fp32
    with np.errstate(all="ignore"):
        max1 = logits.max(-1, keepdims=True)
        ind1 = logits.argmax(-1)
        factor1 = np.maximum(np.abs(logits), max1)
        mask1 = (max1 - logits) / factor1 > 2.0 * JITTER_EPS
        m1 = np.where(mask1, -np.inf, logits)
        e1 = np.exp(m1 - max1)
        g1 = e1 / e1.sum(-1, keepdims=True)
        mult1 = np.take_along_axis(g1, ind1[:, None], -1)[:, 0]
        oh1 = np.arange(E)[None, :] == ind1[:, None]
        ms = np.where(oh1, -np.inf, logits)
        max2 = ms.max(-1, keepdims=True)
        ind2 = ms.argmax(-1)
        factor2 = np.maximum(np.abs(logits), max2)
        mask2 = (max2 - logits) / factor2 > 2.0 * JITTER_EPS
        m2 = np.where(mask2, -np.inf, ms)
        e2 = np.exp(m2 - max2)
        g2 = e2 / e2.sum(-1, keepdims=True)
        mult2 = np.take_along_axis(g2, ind2[:, None], -1)[:, 0]
    return logits, ind1, mult1.astype(np.float32), ind2, mult2.astype(np.float32)


def _tile_weights(w_gate_e, w_up_e, w_down_e):
    # wg/wu: [F, D] -> [16(ft), 128(p), 8(k), 128(c)]; tile[p, k*128+c] = w[ft*128+c, k*128+p]
    wgt = np.ascontiguousarray(
        w_gate_e.reshape(16, 128, 8, 128).transpose(0, 3, 2, 1)).reshape(16, 128, 1024)
    wut = np.ascontiguousarray(
        w_up_e.reshape(16, 128, 8, 128).transpose(0, 3, 2, 1)).reshape(16, 128, 1024)
    # wd: [D, F] -> [8(dt), 128(p), 16(k2), 128(c)]; tile[p, k2*128+c] = w[dt*128+c, k2*128+p]
    wdt = np.ascontiguousarray(
        w_down_e.reshape(8, 128, 16, 128).transpose(0, 3, 2, 1)).reshape(8, 128, 2048)
    return wgt, wut, wdt


def _silu_ffn(x_rows, wg_e, wu_e, wd_e):
    g = x_rows @ wg_e.T
    u = x_rows @ wu_e.T
    h = g / (1.0 + np.exp(-g)) * u
    return h @ wd_e.T


def kernel(hidden_states, gate_w, w_gate, w_up, w_down):
    from concourse.bass_utils import run_bass_kernel_spmd

    x = np.ascontiguousarray(hidden_states.reshape(-1, D)).astype(np.float32, copy=False)
    logits, ind1, mult1, ind2, mult2 = _route(x, gate_w.astype(np.float32, copy=False))

    dev_idx = []
    dev_w = []
    ov = []  # (e, idx, w)
    for e in range(E):
        t1 = np.nonzero(ind1 == e)[0]
        t2 = np.nonzero(ind2 == e)[0]
        idx = np.concatenate([t1, t2])
        w = np.concatenate([mult1[t1], mult2[t2]])
        if len(idx) > CAP:
            ov.append((e, idx[CAP:], w[CAP:]))
            idx, w = idx[:CAP], w[:CAP]
        dev_idx.append(idx)
        dev_w.append(w)

    tp = TP_DEFAULT
    nc = _get_nc(tp)

    in_maps = []
    for e in range(E):
        idx = dev_idx[e]
        n = len(idx)
        xg_full = np.zeros((tp, D), np.float32)
        xg_full[:n] = x[idx]
        x_tiled = np.ascontiguousarray(xg_full.reshape(tp, 8, 128).transpose(1, 2, 0))
        cbv = np.zeros(tp, np.float32)
        cbv[:n] = dev_w[e]
        cb_tiled = np.ascontiguousarray(np.broadcast_to(cbv, (128, tp)))
        wgt, wut, wdt = _tile_weights(
            np.asarray(w_gate[e], np.float32),
            np.asarray(w_up[e], np.float32),
            np.asarray(w_down[e], np.float32))
        in_maps.append({"xg": x_tiled, "wg": wgt, "wu": wut, "wd": wdt, "cb": cb_tiled})

    res = run_bass_kernel_spmd(nc, in_maps, list(range(8)))

    out = np.zeros((T, D), np.float32)
    for e in range(E):
        n = len(dev_idx[e])
        ye = np.asarray(res.results[e]["y"])  # [D, tp]
        out[dev_idx[e]] += ye[:, :n].T
    for (e, idx, w) in ov:
        yo = _silu_ffn(x[idx],
                       np.asarray(w_gate[e], np.float32),
                       np.asarray(w_up[e], np.float32),
                       np.asarray(w_down[e], np.float32))
        out[idx] += w[:, None] * yo
    return (np.ascontiguousarray(out.reshape(2, 2048, D), dtype=np.float32),
            logits.astype(np.float32, copy=False))


# revision 11
# speedup vs baseline: 1.1385x; 1.0235x over previous
import numpy as np

E, D, F, T = 8, 1024, 2048, 4096
JITTER_EPS = 0.01
TP_DEFAULT = 1024
CAP = 1024

_CACHE = {}


def _token_tiles(tp):
    tiles = []
    off = 0
    rem = tp
    while rem > 0:
        if rem > 512 and rem - 512 < 256:
            sz = rem - 256
        else:
            sz = min(512, rem)
        tiles.append((off, sz))
        off += sz
        rem -= sz
    return tiles


def _build(tp):
    from concourse import bacc, mybir, tile

    tiles = _token_tiles(tp)
    nc = bacc.Bacc("TRN2", target_bir_lowering=False, debug=False, num_devices=8)
    f32 = mybir.dt.float32
    f32r = mybir.dt.float32r
    xg = nc.dram_tensor("xg", [8, 128, tp], f32r, kind="ExternalInput").ap()
    wg = nc.dram_tensor("wg", [16, 128, 1024], f32r, kind="ExternalInput").ap()
    wu = nc.dram_tensor("wu", [16, 128, 1024], f32r, kind="ExternalInput").ap()
    wd = nc.dram_tensor("wd", [8, 128, 2048], f32r, kind="ExternalInput").ap()
    cb = nc.dram_tensor("cb", [128, tp], f32, kind="ExternalInput").ap()
    y = nc.dram_tensor("y", [1024, tp], f32, kind="ExternalOutput").ap()

    with tile.TileContext(nc) as tc:
        with tc.tile_pool(name="xp", bufs=1) as xp, \
             tc.tile_pool(name="wp", bufs=2) as wp, \
             tc.tile_pool(name="wdp", bufs=2) as wdp, \
             tc.tile_pool(name="hp", bufs=1) as hp, \
             tc.tile_pool(name="cp", bufs=1) as cp, \
             tc.tile_pool(name="sp", bufs=2) as spool, \
             tc.tile_pool(name="yp", bufs=2) as yp, \
             tc.tile_pool(name="psA", bufs=2, space="PSUM") as psA, \
             tc.tile_pool(name="psB", bufs=2, space="PSUM") as psB:
            # first gate/up weight tiles go out on the sync queue before x
            wgt_c = wp.tile([128, 1024], f32r, name="wgt")
            wut_c = wp.tile([128, 1024], f32r, name="wut")
            nc.sync.dma_start(wgt_c[:], wg[0, :, :])
            nc.sync.dma_start(wut_c[:], wu[0, :, :])

            # x split across both queues so the PE can track arrival
            x_sb = [xp.tile([128, tp], f32r, name=f"x_{k}") for k in range(8)]
            for k in (0, 2, 4, 6):
                nc.scalar.dma_start(x_sb[k][:], xg[k, :, :])
            for k in (1, 3, 5, 7):
                nc.sync.dma_start(x_sb[k][:], xg[k, :, :])
            cb_sb = cp.tile([128, tp], f32, name="cb_sb")
            nc.scalar.dma_start(cb_sb[:], cb[:, :])

            h_sb = [hp.tile([128, tp], f32r, name=f"h_{i}") for i in range(16)]

            for ft in range(16):
                wgt, wut = wgt_c, wut_c
                if ft + 1 < 16:
                    wgt_c = wp.tile([128, 1024], f32r, name="wgt")
                    wut_c = wp.tile([128, 1024], f32r, name="wut")
                    nc.sync.dma_start(wgt_c[:], wg[ft + 1, :, :])
                    nc.sync.dma_start(wut_c[:], wu[ft + 1, :, :])
                if ft == 0:
                    # k-outer: consume each x_sb[k] as it lands
                    pgs = [psA.tile([128, 512], f32, name="pg") for _ in tiles]
                    pus = [psA.tile([128, 512], f32, name="pu") for _ in tiles]
                    for k in range(8):
                        for t, (toff, tsz) in enumerate(tiles):
                            nc.tensor.matmul(pgs[t][:, :tsz],
                                             wgt[:, k * 128:(k + 1) * 128],
                                             x_sb[k][:, toff:toff + tsz],
                                             start=(k == 0), stop=(k == 7),
                                             skip_group_check=True)
                        for t, (toff, tsz) in enumerate(tiles):
                            nc.tensor.matmul(pus[t][:, :tsz],
                                             wut[:, k * 128:(k + 1) * 128],
                                             x_sb[k][:, toff:toff + tsz],
                                             start=(k == 0), stop=(k == 7),
                                             skip_group_check=True)
                    for t, (toff, tsz) in enumerate(tiles):
                        sg = spool.tile([128, 512], f32, name="sg")
                        nc.scalar.activation(sg[:, :tsz], pgs[t][:, :tsz],
                                             mybir.ActivationFunctionType.Sigmoid)
                        nc.vector.tensor_tensor(sg[:, :tsz], sg[:, :tsz],
                                                pgs[t][:, :tsz], mybir.AluOpType.mult)
                        nc.vector.tensor_tensor(h_sb[0][:, toff:toff + tsz],
                                                sg[:, :tsz], pus[t][:, :tsz],
                                                mybir.AluOpType.mult)
                    continue
                for (toff, tsz) in tiles:
                    pg = psA.tile([128, 512], f32, name="pg")
                    pu = psA.tile([128, 512], f32, name="pu")
                    for k in range(8):
                        nc.tensor.matmul(pg[:, :tsz], wgt[:, k * 128:(k + 1) * 128],
                                         x_sb[k][:, toff:toff + tsz],
                                         start=(k == 0), stop=(k == 7))
                    for k in range(8):
                        nc.tensor.matmul(pu[:, :tsz], wut[:, k * 128:(k + 1) * 128],
                                         x_sb[k][:, toff:toff + tsz],
                                         start=(k == 0), stop=(k == 7))
                    sg = spool.tile([128, 512], f32, name="sg")
                    nc.scalar.activation(sg[:, :tsz], pg[:, :tsz],
                                         mybir.ActivationFunctionType.Sigmoid)
                    nc.vector.tensor_tensor(sg[:, :tsz], sg[:, :tsz], pg[:, :tsz],
                                            mybir.AluOpType.mult)
                    nc.vector.tensor_tensor(h_sb[ft][:, toff:toff + tsz], sg[:, :tsz],
                                            pu[:, :tsz], mybir.AluOpType.mult)

            # last group uses finer tiles so the exposed epilogue is short
            last_tiles = list(tiles[:-1])
            loff, lsz = tiles[-1]
            h1 = lsz // 2
            last_tiles += [(loff, h1), (loff + h1, lsz - h1)]

            wdt_c = wdp.tile([128, 2048], f32r, name="wdt")
            nc.sync.dma_start(wdt_c[:], wd[0, :, :])
            for dt_ in range(8):
                wdt = wdt_c
                if dt_ + 1 < 8:
                    wdt_c = wdp.tile([128, 2048], f32r, name="wdt")
                    nc.sync.dma_start(wdt_c[:], wd[dt_ + 1, :, :])
                for (toff, tsz) in (tiles if dt_ < 7 else last_tiles):
                    py_ = psB.tile([128, 512], f32, name="py")
                    for k2 in range(16):
                        nc.tensor.matmul(py_[:, :tsz], wdt[:, k2 * 128:(k2 + 1) * 128],
                                         h_sb[k2][:, toff:toff + tsz],
                                         start=(k2 == 0), stop=(k2 == 15))
                    ysb = yp.tile([128, 512], f32, name="ysb")
                    nc.vector.tensor_tensor(ysb[:, :tsz], py_[:, :tsz],
                                            cb_sb[:, toff:toff + tsz],
                                            mybir.AluOpType.mult)
                    nc.scalar.dma_start(y[dt_ * 128:(dt_ + 1) * 128, toff:toff + tsz],
                                        ysb[:, :tsz])
    nc.compile()
    return nc


def _get_nc(tp):
    if tp not in _CACHE:
        _CACHE[tp] = _build(tp)
    return _CACHE[tp]


def _route(x, gate_w):
    logits = x @ gate_w.T  # [T, E] fp32
    with np.errstate(all="ignore"):
        max1 = logits.max(-1, keepdims=True)
        ind1 = logits.argmax(-1)
        factor1 = np.maximum(np.abs(logits), max1)
        mask1 = (max1 - logits) / factor1 > 2.0 * JITTER_EPS
        m1 = np.where(mask1, -np.inf, logits)
        e1 = np.exp(m1 - max1)
        g1 = e1 / e1.sum(-1, keepdims=True)
        mult1 = np.take_along_axis(g1, ind1[:, None], -1)[:, 0]
        oh1 = np.arange(E)[None, :] == ind1[:, None]
        ms = np.where(oh1, -np.inf, logits)
        max2 = ms.max(-1, keepdims=True)
        ind2 = ms.argmax(-1)
        factor2 = np.maximum(np.abs(logits), max2)
        mask2 = (max2 - logits) / factor2 > 2.0 * JITTER_EPS
        m2 = np.where(mask2, -np.inf, ms)
        e2 = np.exp(m2 - max2)
        g2 = e2 / e2.sum(-1, keepdims=True)
        mult2 = np.take_along_axis(g2, ind2[:, None], -1)[:, 0]
    return logits, ind1, mult1.astype(np.float32), ind2, mult2.astype(np.float32)


def _tile_weights(w_gate_e, w_up_e, w_down_e):
    # wg/wu: [F, D] -> [16(ft), 128(p), 8(k), 128(c)]; tile[p, k*128+c] = w[ft*128+c, k*128+p]
    wgt = np.ascontiguousarray(
        w_gate_e.reshape(16, 128, 8, 128).transpose(0, 3, 2, 1)).reshape(16, 128, 1024)
    wut = np.ascontiguousarray(
        w_up_e.reshape(16, 128, 8, 128).transpose(0, 3, 2, 1)).reshape(16, 128, 1024)
    # wd: [D, F] -> [8(dt), 128(p), 16(k2), 128(c)]; tile[p, k2*128+c] = w[dt*128+c, k2*128+p]
    wdt = np.ascontiguousarray(
        w_down_e.reshape(8, 128, 16, 128).transpose(0, 3, 2, 1)).reshape(8, 128, 2048)
    return wgt, wut, wdt


def _silu_ffn(x_rows, wg_e, wu_e, wd_e):
    g = x_rows @ wg_e.T
    u = x_rows @ wu_e.T
    h = g / (1.0 + np.exp(-g)) * u
    return h @ wd_e.T


def kernel(hidden_states, gate_w, w_gate, w_up, w_down):
    from concourse.bass_utils import run_bass_kernel_spmd

    x = np.ascontiguousarray(hidden_states.reshape(-1, D)).astype(np.float32, copy=False)
    logits, ind1, mult1, ind2, mult2 = _route(x, gate_w.astype(np.float32, copy=False))

    dev_idx = []
    dev_w = []
    ov = []  # (e, idx, w)
    for e in range(E):
        t1 = np.nonzero(ind1 == e)[0]
        t2 = np.nonzero(ind2 == e)[0]
        idx = np.concatenate([t1, t2])
        w = np.concatenate([mult1[t1], mult2[t2]])
        if len(idx) > CAP:
            ov.append((e, idx[CAP:], w[CAP:]))
            idx, w = idx[:CAP], w[:CAP]
        dev_idx.append(idx)
        dev_w.append(w)

    tp = TP_DEFAULT
    nc = _get_nc(tp)

    in_maps = []
    for e in range(E):
        idx = dev_idx[e]
        n = len(idx)
        xg_full = np.zeros((tp, D), np.float32)
        xg_full[:n] = x[idx]
        x_tiled = np.ascontiguousarray(xg_full.reshape(tp, 8, 128).transpose(1, 2, 0))
        cbv = np.zeros(tp, np.float32)
        cbv[:n] = dev_w[e]
        cb_tiled = np.ascontiguousarray(np.broadcast_to(cbv, (128, tp)))
        wgt, wut, wdt = _tile_weights(
            np.asarray(w_gate[e], np.float32),
            np.asarray(w_up[e], np.float32),
            np.asarray(w_down[e], np.float32))
        in_maps.append({"xg": x_tiled, "wg": wgt, "wu": wut, "wd": wdt, "cb": cb_tiled})

    res = run_bass_kernel_spmd(nc, in_maps, list(range(8)))

    out = np.zeros((T, D), np.float32)
    for e in range(E):
        n = len(dev_idx[e])
        ye = np.asarray(res.results[e]["y"])  # [D, tp]
        out[dev_idx[e]] += ye[:, :n].T
    for (e, idx, w) in ov:
        yo = _silu_ffn(x[idx],
                       np.asarray(w_gate[e], np.float32),
                       np.asarray(w_up[e], np.float32),
                       np.asarray(w_down[e], np.float32))
        out[idx] += w[:, None] * yo
    return (np.ascontiguousarray(out.reshape(2, 2048, D), dtype=np.float32),
            logits.astype(np.float32, copy=False))


# revision 13
# speedup vs baseline: 1.1581x; 1.0172x over previous
import numpy as np

E, D, F, T = 8, 1024, 2048, 4096
JITTER_EPS = 0.01
TP_DEFAULT = 1024
CAP = 1024

_CACHE = {}


def _token_tiles(tp):
    tiles = []
    off = 0
    rem = tp
    while rem > 0:
        if rem > 512 and rem - 512 < 256:
            sz = rem - 256
        else:
            sz = min(512, rem)
        tiles.append((off, sz))
        off += sz
        rem -= sz
    return tiles


def _build(tp):
    from concourse import bacc, mybir, tile

    tiles = _token_tiles(tp)
    nc = bacc.Bacc("TRN2", target_bir_lowering=False, debug=False, num_devices=8)
    f32 = mybir.dt.float32
    f32r = mybir.dt.float32r
    xg = nc.dram_tensor("xg", [8, 128, tp], f32r, kind="ExternalInput").ap()
    wg = nc.dram_tensor("wg", [16, 128, 1024], f32r, kind="ExternalInput").ap()
    wu = nc.dram_tensor("wu", [16, 128, 1024], f32r, kind="ExternalInput").ap()
    wd = nc.dram_tensor("wd", [8, 128, 2048], f32r, kind="ExternalInput").ap()
    cb = nc.dram_tensor("cb", [128, tp], f32, kind="ExternalInput").ap()
    y = nc.dram_tensor("y", [1024, tp], f32, kind="ExternalOutput").ap()

    with tile.TileContext(nc) as tc:
        with tc.tile_pool(name="xp", bufs=1) as xp, \
             tc.tile_pool(name="wp", bufs=2) as wp, \
             tc.tile_pool(name="wdp", bufs=3) as wdp, \
             tc.tile_pool(name="hp", bufs=1) as hp, \
             tc.tile_pool(name="cp", bufs=1) as cp, \
             tc.tile_pool(name="sp", bufs=2) as spool, \
             tc.tile_pool(name="yp", bufs=2) as yp, \
             tc.tile_pool(name="psA", bufs=2, space="PSUM") as psA, \
             tc.tile_pool(name="psB", bufs=2, space="PSUM") as psB:
            # first gate/up weight tiles go out on the sync queue before x
            wgt_c = wp.tile([128, 1024], f32r, name="wgt")
            wut_c = wp.tile([128, 1024], f32r, name="wut")
            nc.sync.dma_start(wgt_c[:], wg[0, :, :])
            nc.sync.dma_start(wut_c[:], wu[0, :, :])

            # x split across both queues so the PE can track arrival
            x_sb = [xp.tile([128, tp], f32r, name=f"x_{k}") for k in range(8)]
            for k in (0, 2, 4, 6):
                nc.scalar.dma_start(x_sb[k][:], xg[k, :, :])
            for k in (1, 3, 5, 7):
                nc.sync.dma_start(x_sb[k][:], xg[k, :, :])
            cb_sb = cp.tile([128, tp], f32, name="cb_sb")
            nc.scalar.dma_start(cb_sb[:], cb[:, :])

            h_sb = [hp.tile([128, tp], f32r, name=f"h_{i}") for i in range(16)]

            for ft in range(16):
                wgt, wut = wgt_c, wut_c
                if ft + 1 < 16:
                    wgt_c = wp.tile([128, 1024], f32r, name="wgt")
                    wut_c = wp.tile([128, 1024], f32r, name="wut")
                    nc.sync.dma_start(wgt_c[:], wg[ft + 1, :, :])
                    nc.sync.dma_start(wut_c[:], wu[ft + 1, :, :])
                if ft == 0:
                    # k-outer: consume each x_sb[k] as it lands
                    pgs = [psA.tile([128, 512], f32, name="pg") for _ in tiles]
                    pus = [psA.tile([128, 512], f32, name="pu") for _ in tiles]
                    for k in range(8):
                        for t, (toff, tsz) in enumerate(tiles):
                            nc.tensor.matmul(pgs[t][:, :tsz],
                                             wgt[:, k * 128:(k + 1) * 128],
                                             x_sb[k][:, toff:toff + tsz],
                                             start=(k == 0), stop=(k == 7),
                                             skip_group_check=True)
                        for t, (toff, tsz) in enumerate(tiles):
                            nc.tensor.matmul(pus[t][:, :tsz],
                                             wut[:, k * 128:(k + 1) * 128],
                                             x_sb[k][:, toff:toff + tsz],
                                             start=(k == 0), stop=(k == 7),
                                             skip_group_check=True)
                    for t, (toff, tsz) in enumerate(tiles):
                        sg = spool.tile([128, 512], f32, name="sg")
                        nc.scalar.activation(sg[:, :tsz], pgs[t][:, :tsz],
                                             mybir.ActivationFunctionType.Sigmoid)
                        nc.vector.tensor_tensor(sg[:, :tsz], sg[:, :tsz],
                                                pgs[t][:, :tsz], mybir.AluOpType.mult)
                        nc.vector.tensor_tensor(h_sb[0][:, toff:toff + tsz],
                                                sg[:, :tsz], pus[t][:, :tsz],
                                                mybir.AluOpType.mult)
                    continue
                for (toff, tsz) in tiles:
                    pg = psA.tile([128, 512], f32, name="pg")
                    pu = psA.tile([128, 512], f32, name="pu")
                    for k in range(8):
                        nc.tensor.matmul(pg[:, :tsz], wgt[:, k * 128:(k + 1) * 128],
                                         x_sb[k][:, toff:toff + tsz],
                                         start=(k == 0), stop=(k == 7))
                    for k in range(8):
                        nc.tensor.matmul(pu[:, :tsz], wut[:, k * 128:(k + 1) * 128],
                                         x_sb[k][:, toff:toff + tsz],
                                         start=(k == 0), stop=(k == 7))
                    sg = spool.tile([128, 512], f32, name="sg")
                    nc.scalar.activation(sg[:, :tsz], pg[:, :tsz],
                                         mybir.ActivationFunctionType.Sigmoid)
                    nc.vector.tensor_tensor(sg[:, :tsz], sg[:, :tsz], pg[:, :tsz],
                                            mybir.AluOpType.mult)
                    nc.vector.tensor_tensor(h_sb[ft][:, toff:toff + tsz], sg[:, :tsz],
                                            pu[:, :tsz], mybir.AluOpType.mult)

            # last group uses finer tiles so the exposed epilogue is short
            last_tiles = list(tiles[:-1])
            loff, lsz = tiles[-1]
            h1 = lsz // 2
            last_tiles += [(loff, h1), (loff + h1, lsz - h1)]

            wd_q = []
            for j in range(2):
                t_ = wdp.tile([128, 2048], f32r, name="wdt")
                nc.sync.dma_start(t_[:], wd[j, :, :])
                wd_q.append(t_)
            for dt_ in range(8):
                wdt = wd_q[dt_]
                if dt_ + 2 < 8:
                    t_ = wdp.tile([128, 2048], f32r, name="wdt")
                    nc.sync.dma_start(t_[:], wd[dt_ + 2, :, :])
                    wd_q.append(t_)
                for (toff, tsz) in (tiles if dt_ < 7 else last_tiles):
                    py_ = psB.tile([128, 512], f32, name="py")
                    for k2 in range(16):
                        nc.tensor.matmul(py_[:, :tsz], wdt[:, k2 * 128:(k2 + 1) * 128],
                                         h_sb[k2][:, toff:toff + tsz],
                                         start=(k2 == 0), stop=(k2 == 15))
                    ysb = yp.tile([128, 512], f32, name="ysb")
                    nc.vector.tensor_tensor(ysb[:, :tsz], py_[:, :tsz],
                                            cb_sb[:, toff:toff + tsz],
                                            mybir.AluOpType.mult)
                    nc.scalar.dma_start(y[dt_ * 128:(dt_ + 1) * 128, toff:toff + tsz],
                                        ysb[:, :tsz])
    nc.compile()
    return nc


def _get_nc(tp):
    if tp not in _CACHE:
        _CACHE[tp] = _build(tp)
    return _CACHE[tp]


def _route(x, gate_w):
    logits = x @ gate_w.T  # [T, E] fp32
    with np.errstate(all="ignore"):
        max1 = logits.max(-1, keepdims=True)
        ind1 = logits.argmax(-1)
        factor1 = np.maximum(np.abs(logits), max1)
        mask1 = (max1 - logits) / factor1 > 2.0 * JITTER_EPS
        m1 = np.where(mask1, -np.inf, logits)
        e1 = np.exp(m1 - max1)
        g1 = e1 / e1.sum(-1, keepdims=True)
        mult1 = np.take_along_axis(g1, ind1[:, None], -1)[:, 0]
        oh1 = np.arange(E)[None, :] == ind1[:, None]
        ms = np.where(oh1, -np.inf, logits)
        max2 = ms.max(-1, keepdims=True)
        ind2 = ms.argmax(-1)
        factor2 = np.maximum(np.abs(logits), max2)
        mask2 = (max2 - logits) / factor2 > 2.0 * JITTER_EPS
        m2 = np.where(mask2, -np.inf, ms)
        e2 = np.exp(m2 - max2)
        g2 = e2 / e2.sum(-1, keepdims=True)
        mult2 = np.take_along_axis(g2, ind2[:, None], -1)[:, 0]
    return logits, ind1, mult1.astype(np.float32), ind2, mult2.astype(np.float32)


def _tile_weights(w_gate_e, w_up_e, w_down_e):
    # wg/wu: [F, D] -> [16(ft), 128(p), 8(k), 128(c)]; tile[p, k*128+c] = w[ft*128+c, k*128+p]
    wgt = np.ascontiguousarray(
        w_gate_e.reshape(16, 128, 8, 128).transpose(0, 3, 2, 1)).reshape(16, 128, 1024)
    wut = np.ascontiguousarray(
        w_up_e.reshape(16, 128, 8, 128).transpose(0, 3, 2, 1)).reshape(16, 128, 1024)
    # wd: [D, F] -> [8(dt), 128(p), 16(k2), 128(c)]; tile[p, k2*128+c] = w[dt*128+c, k2*128+p]
    wdt = np.ascontiguousarray(
        w_down_e.reshape(8, 128, 16, 128).transpose(0, 3, 2, 1)).reshape(8, 128, 2048)
    return wgt, wut, wdt


def _silu_ffn(x_rows, wg_e, wu_e, wd_e):
    g = x_rows @ wg_e.T
    u = x_rows @ wu_e.T
    h = g / (1.0 + np.exp(-g)) * u
    return h @ wd_e.T


def kernel(hidden_states, gate_w, w_gate, w_up, w_down):
    from concourse.bass_utils import run_bass_kernel_spmd

    x = np.ascontiguousarray(hidden_states.reshape(-1, D)).astype(np.float32, copy=False)
    logits, ind1, mult1, ind2, mult2 = _route(x, gate_w.astype(np.float32, copy=False))

    dev_idx = []
    dev_w = []
    ov = []  # (e, idx, w)
    for e in range(E):
        t1 = np.nonzero(ind1 == e)[0]
        t2 = np.nonzero(ind2 == e)[0]
        idx = np.concatenate([t1, t2])
        w = np.concatenate([mult1[t1], mult2[t2]])
        if len(idx) > CAP:
            ov.append((e, idx[CAP:], w[CAP:]))
            idx, w = idx[:CAP], w[:CAP]
        dev_idx.append(idx)
        dev_w.append(w)

    tp = TP_DEFAULT
    nc = _get_nc(tp)

    in_maps = []
    for e in range(E):
        idx = dev_idx[e]
        n = len(idx)
        xg_full = np.zeros((tp, D), np.float32)
        xg_full[:n] = x[idx]
        x_tiled = np.ascontiguousarray(xg_full.reshape(tp, 8, 128).transpose(1, 2, 0))
        cbv = np.zeros(tp, np.float32)
        cbv[:n] = dev_w[e]
        cb_tiled = np.ascontiguousarray(np.broadcast_to(cbv, (128, tp)))
        wgt, wut, wdt = _tile_weights(
            np.asarray(w_gate[e], np.float32),
            np.asarray(w_up[e], np.float32),
            np.asarray(w_down[e], np.float32))
        in_maps.append({"xg": x_tiled, "wg": wgt, "wu": wut, "wd": wdt, "cb": cb_tiled})

    res = run_bass_kernel_spmd(nc, in_maps, list(range(8)))

    out = np.zeros((T, D), np.float32)
    for e in range(E):
        n = len(dev_idx[e])
        ye = np.asarray(res.results[e]["y"])  # [D, tp]
        out[dev_idx[e]] += ye[:, :n].T
    for (e, idx, w) in ov:
        yo = _silu_ffn(x[idx],
                       np.asarray(w_gate[e], np.float32),
                       np.asarray(w_up[e], np.float32),
                       np.asarray(w_down[e], np.float32))
        out[idx] += w[:, None] * yo
    return (np.ascontiguousarray(out.reshape(2, 2048, D), dtype=np.float32),
            logits.astype(np.float32, copy=False))
